# revision 3
# baseline (speedup 1.0000x reference)
"""Trainium2 Bass kernel for nn_PositiveSlopeLinearLoss.

Computation (see the PyTorch/JAX reference):
  v = x0_vals[:, 3]                                  # (N,) electronegativity
  per-segment (molecule) mean of v, then
  diff_en[s] = 4 * sum_{i in s} |v_i - mean_s|       # (B,)
  ... then a 1-D least-squares fit of y_pred vs diff_en and a few scalar
  penalties, all collapsing to a single f32 scalar.

Device does the memory-bound part: stream the 256 MB x0_vals, and per
molecule (512 consecutive atoms) produce sum_v and sum|v - mean|.
Everything after diff_en is O(B)=16K scalar moment math, done on host.

Sharding: data-parallel across 8 NeuronCores; core k takes molecules
[k*2048, (k+1)*2048) == atom rows [k*1048576, (k+1)*1048576).
"""

import sys

import numpy as np

sys.path.insert(0, "/opt/trn_rl_repo")

B = 16384  # molecules
L = 512  # atoms per molecule (uniform fast path)
F = 8  # features per atom; column 3 is used
N = B * L
NCORES = 8
B_CORE = B // NCORES  # 2048 molecules per core
N_CORE = B_CORE * L  # 1048576 atom rows per core
M = 2  # molecules per partition per tile
T = B_CORE // (128 * M)  # tiles per core
PARAMS = (0.6, 0.3, 0.8)

_CACHE = {}


def _build_bass():
    from concourse import bacc, mybir
    from concourse.tile import TileContext

    nc = bacc.Bacc()
    x0 = nc.dram_tensor("x0", [N_CORE, F], mybir.dt.float32, kind="ExternalInput")
    out = nc.dram_tensor("diff", [128, T * M], mybir.dt.float32, kind="ExternalOutput")

    # tile t, partition p, slot m  <->  molecule (t*128 + p)*M + m of this core
    x0r = x0.rearrange("(t p m l) f -> t p (m l f)", t=T, p=128, m=M, l=L)

    with TileContext(nc) as tc:
        with (
            tc.tile_pool(name="raw", bufs=3) as raw_pool,
            tc.tile_pool(name="stats", bufs=1) as stats_pool,
        ):
            negsum = stats_pool.tile([128, T * M], mybir.dt.float32)
            absdev = stats_pool.tile([128, T * M], mybir.dt.float32)
            for t in range(T):
                rt = raw_pool.tile([128, M * L * F], mybir.dt.float32, tag="raw")
                nc.sync.dma_start(out=rt[:], in_=x0r[t])
                # view: [p, m, f, l] with l innermost (stride F)
                vv = rt[:].rearrange("p (m l f) -> p m f l", m=M, l=L, f=F)
                v = vv[:, :, 3:4, :]  # [128, M, 1, L]
                # negsum[:, t*M+m] = -sum_l v[p, m, l]
                nc.vector.reduce_sum(
                    negsum[:, t * M : (t + 1) * M],
                    v,
                    axis=mybir.AxisListType.X,
                    negate=True,
                )
                for m in range(M):
                    c = t * M + m
                    vm = vv[:, m : m + 1, 3:4, :]  # [128,1,1,L]
                    # in-place: |L*v - sum| ; accum_out = per-partition sum
                    nc.scalar.activation(
                        out=vm,
                        in_=vm,
                        func=mybir.ActivationFunctionType.Abs,
                        bias=negsum[:, c : c + 1],
                        scale=float(L),
                        accum_out=absdev[:, c : c + 1],
                    )
            nc.sync.dma_start(out=out[:, :], in_=absdev[:])
    nc.finalize()
    return nc


def _scalar_tail(diff_en, y_pred, y):
    """Everything after diff_en, replicated from the reference in float64."""
    x = diff_en.astype(np.float64)
    yp = y_pred.astype(np.float64)
    yt = y.astype(np.float64)
    mx = x.mean()
    my = yp.mean()
    xc = x - mx
    m = (xc * (yp - my)).mean() / (xc * xc).mean()
    c = my - m * mx
    r = yp - (m * x + c)
    p0, p1, p2 = PARAMS
    residual_penalty = (p0 * r * r + 1.0).mean()
    slope_penalty = np.logaddexp(0.0, p2 * (p1 * -m)) / p2 + 1.0
    d = yp - yt
    ad = np.abs(d)
    sl1 = np.where(ad < 1.0, 0.5 * d * d, ad - 0.5).mean()
    return np.float32(sl1 * residual_penalty * slope_penalty)


def _host_fallback(x0_vals, segment_ids, y_pred, y):
    """Pure-numpy path for non-uniform (ragged) segment_ids."""
    v = x0_vals[:, 3].astype(np.float64)
    seg = segment_ids.astype(np.int64)
    counts = np.bincount(seg, minlength=B)
    sums = np.bincount(seg, weights=v, minlength=B)
    means = np.divide(sums, counts, out=np.zeros(B), where=counts > 0)
    dev = np.abs(v - means[seg])
    diff_en = 4.0 * np.bincount(seg, weights=dev, minlength=B)
    return _scalar_tail(diff_en, y_pred, y)


def _run_device(x0_vals):
    from concourse.bass_utils import run_bass_kernel_spmd

    if "nc" not in _CACHE:
        _CACHE["nc"] = _build_bass()
    nc = _CACHE["nc"]

    chunks = x0_vals.reshape(NCORES, N_CORE, F)
    in_maps = [{"x0": np.ascontiguousarray(chunks[i])} for i in range(NCORES)]
    res = run_bass_kernel_spmd(nc, in_maps, core_ids=list(range(NCORES)))
    per_core = []
    for i in range(NCORES):
        d = res.results[i]["diff"]  # (128, T*M); molecule = (t*128+p)*M + m
        per_core.append(d.reshape(128, T, M).transpose(1, 0, 2).reshape(-1))
    absdev = np.concatenate(per_core)  # (B,) == L * sum|v - mean|
    return absdev * (4.0 / L)


def kernel(x0_vals, segment_ids, y_pred, y):
    expected = np.repeat(np.arange(B, dtype=segment_ids.dtype), L)
    if not np.array_equal(segment_ids, expected):
        return _host_fallback(x0_vals, segment_ids, y_pred, y)
    diff_en = _run_device(np.asarray(x0_vals, dtype=np.float32))
    return _scalar_tail(diff_en, y_pred, y)


# revision 5
# speedup vs baseline: 1.0329x; 1.0329x over previous
"""Trainium2 Bass kernel for nn_PositiveSlopeLinearLoss.

Math (mirrors the JAX reference):
  v = x0_vals[:, 3]
  per-molecule (512 sorted, contiguous atoms) mean of v, then
  diff_en[s] = 4 * sum_{i in s} |v_i - mean_s|
  ... followed by a 1-D least-squares fit of y_pred vs diff_en plus scalar
  penalties, collapsing to one f32 scalar. Everything after diff_en is
  O(B)=16K scalar math, done on host in float64.

Device part (the memory-bound 256 MB stream) is data-parallel across the
8 NeuronCores: core k takes molecules [k*2048, (k+1)*2048). Per core a
raw-bacc pipeline streams 32 MB over two HWDGE rings (SP ring: even
tiles, ACT ring: odd tiles — balances all 16 SDMA engines), and per tile:
  DVE:  reduce_sum(negate) over the stride-8 column  -> -sum_v
  ACT:  Abs(L*v - sum_v) in place, accum_out         -> L*sum|v - mean|
Output per core: 2048 f32 (8 KB). Host scales by 4/L and computes the
final scalar. measured ~95 us/core HW time (32 MB read; ~89.4 us HBM
roofline at 358 GB/s).
"""

import sys

import numpy as np

sys.path.insert(0, "/opt/trn_rl_repo")

B = 16384  # molecules
L = 512  # atoms per molecule (uniform fast path)
F = 8  # features per atom; column 3 is used
N = B * L
NCORES = 8
B_CORE = B // NCORES  # 2048 molecules per core
N_CORE = B_CORE * L  # 1048576 atom rows per core
M = 1  # molecules per partition per tile
BUFS = 10  # SBUF slots (16 KB/partition each)
T = B_CORE // (128 * M)  # tiles per core
PARAMS = (0.6, 0.3, 0.8)

_CACHE = {}


def _build_bass():
    from contextlib import ExitStack

    from concourse import bacc, mybir

    nc = bacc.Bacc()
    x0 = nc.dram_tensor("x0", [N_CORE, F], mybir.dt.float32, kind="ExternalInput")
    out = nc.dram_tensor("diff", [128, T * M], mybir.dt.float32, kind="ExternalOutput")
    # tile t, partition p, slot m  <->  molecule (t*128 + p)*M + m of this core
    x0r = x0.rearrange("(t p m l) f -> t p (m l f)", t=T, p=128, m=M, l=L)
    bufs = BUFS

    with ExitStack() as ctx:
        raw = ctx.enter_context(
            nc.sbuf_tensor("raw", [128, bufs, M * L * F], mybir.dt.float32)
        )
        negsum = ctx.enter_context(
            nc.sbuf_tensor("negsum", [128, T * M], mybir.dt.float32)
        )
        absdev = ctx.enter_context(
            nc.sbuf_tensor("absdev", [128, T * M], mybir.dt.float32)
        )
        dma_slot = [ctx.enter_context(nc.semaphore(f"dma_s{s}")) for s in range(bufs)]
        dve_sem = ctx.enter_context(nc.semaphore("dve_sem"))
        act_sem = ctx.enter_context(nc.semaphore("act_sem"))
        out_sem = ctx.enter_context(nc.semaphore("out_sem"))
        block = ctx.enter_context(nc.Block(no_gpsimd_drain=True))

        def v_ap(t):
            s = t % bufs
            return raw[:, s : s + 1, :].rearrange(
                "p s (m l f) -> p (s m) f l", m=M, l=L, f=F
            )[:, :, 3:4, :]

        @block.sync
        def _(sync):
            for t in range(0, T, 2):
                s = t % bufs
                if t >= bufs:
                    sync.wait_ge(act_sem, t - bufs + 1)
                sync.dma_start(out=raw[:, s : s + 1, :], in_=x0r[t]).then_inc(
                    dma_slot[s], 16
                )
            sync.wait_ge(act_sem, T)
            sync.dma_start(out=out[:, :], in_=absdev[:]).then_inc(out_sem, 16)
            sync.wait_ge(out_sem, 16)

        @block.vector
        def _(vector):
            for t in range(T):
                s = t % bufs
                vector.wait_ge(dma_slot[s], 16 * (t // bufs + 1))
                nc.vector.reduce_sum(
                    negsum[:, t * M : (t + 1) * M],
                    v_ap(t),
                    axis=mybir.AxisListType.X,
                    negate=True,
                ).then_inc(dve_sem, 1)

        @block.scalar
        def _(scalar):
            def issue_odd(t):
                s = t % bufs
                if t >= bufs:
                    # the act that freed this slot must have fully retired
                    scalar.wait_ge(act_sem, t - bufs + 1)
                scalar.dma_start(out=raw[:, s : s + 1, :], in_=x0r[t]).then_inc(
                    dma_slot[s], 16
                )

            for t in range(1, min(bufs, T), 2):
                issue_odd(t)
            for t in range(T):
                scalar.wait_ge(dve_sem, t + 1)
                for m in range(M):
                    c = t * M + m
                    vm = v_ap(t)[:, m : m + 1, :, :]
                    inst = nc.scalar.activation(
                        out=vm,
                        in_=vm,
                        func=mybir.ActivationFunctionType.Abs,
                        bias=negsum[:, c : c + 1],
                        scale=float(L),
                        accum_out=absdev[:, c : c + 1],
                    )
                inst.then_inc(act_sem, 1)
                nxt = t + bufs  # tile whose slot this act frees
                if nxt < T and nxt % 2 == 1:
                    issue_odd(nxt)

    nc.finalize()
    return nc


def _scalar_tail(diff_en, y_pred, y):
    """Everything after diff_en, replicated from the reference in float64."""
    x = diff_en.astype(np.float64)
    yp = y_pred.astype(np.float64)
    yt = y.astype(np.float64)
    mx = x.mean()
    my = yp.mean()
    xc = x - mx
    m = (xc * (yp - my)).mean() / (xc * xc).mean()
    c = my - m * mx
    r = yp - (m * x + c)
    p0, p1, p2 = PARAMS
    residual_penalty = (p0 * r * r + 1.0).mean()
    slope_penalty = np.logaddexp(0.0, p2 * (p1 * -m)) / p2 + 1.0
    d = yp - yt
    ad = np.abs(d)
    sl1 = np.where(ad < 1.0, 0.5 * d * d, ad - 0.5).mean()
    return np.asarray(sl1 * residual_penalty * slope_penalty, dtype=np.float32)


def _host_fallback(x0_vals, segment_ids, y_pred, y):
    """Pure-numpy path for non-uniform (ragged) segment_ids."""
    nseg = int(y_pred.shape[0])
    v = x0_vals[:, 3].astype(np.float64)
    seg = segment_ids.astype(np.int64)
    counts = np.bincount(seg, minlength=nseg)
    sums = np.bincount(seg, weights=v, minlength=nseg)
    means = np.divide(sums, counts, out=np.zeros(nseg), where=counts > 0)
    dev = np.abs(v - means[seg])
    diff_en = 4.0 * np.bincount(seg, weights=dev, minlength=nseg)
    return _scalar_tail(diff_en, y_pred, y)


def _run_device(x0_vals):
    from concourse.bass_utils import run_bass_kernel_spmd

    if "nc" not in _CACHE:
        _CACHE["nc"] = _build_bass()
    nc = _CACHE["nc"]

    chunks = x0_vals.reshape(NCORES, N_CORE, F)
    in_maps = [{"x0": np.ascontiguousarray(chunks[i])} for i in range(NCORES)]
    res = run_bass_kernel_spmd(nc, in_maps, core_ids=list(range(NCORES)))
    per_core = []
    for i in range(NCORES):
        d = res.results[i]["diff"]  # (128, T*M); molecule = (t*128+p)*M + m
        per_core.append(d.reshape(128, T, M).transpose(1, 0, 2).reshape(-1))
    absdev = np.concatenate(per_core)  # (B,) == L * sum|v - mean|
    return absdev * (4.0 / L)


def kernel(x0_vals, segment_ids, y_pred, y):
    x0_vals = np.asarray(x0_vals)
    segment_ids = np.asarray(segment_ids)
    y_pred = np.asarray(y_pred)
    y = np.asarray(y)
    expected = np.repeat(
        np.arange(B, dtype=segment_ids.dtype), L
    )
    if (
        x0_vals.shape != (N, F)
        or y_pred.shape != (B,)
        or not np.array_equal(segment_ids, expected)
    ):
        return _host_fallback(x0_vals, segment_ids, y_pred, y)
    diff_en = _run_device(np.ascontiguousarray(x0_vals, dtype=np.float32))
    return _scalar_tail(diff_en, y_pred, y)
